# revision 1
# baseline (speedup 1.0000x reference)
"""DFlashAttention kernel for Trainium2, 8 NeuronCores.

Sharding: 8 cores = 4 batches x 2 KV-head-groups. Each core (b, g) handles
batch b and KV heads [4g, 4g+4) (query heads [16g, 16g+16)), producing the
partial output  sum_{o in group} attn[:, o] @ WoT[o, :]  for its batch. The
host sums the two group partials per batch (row-parallel o-projection).

Per-core device program (SPMD: same NEFF, different input arrays):
  - Q path: q = xn @ WqT (bf16 matmuls, weight stream interleaved with the
    context-piece pipeline), RMSNorm + RoPE on the free axis, PE-transpose
    per head -> qT4[kh] = [128 hd, 4 heads x 64 q] fp32r.
  - Context tokens stream in 256-token pieces through an A/B software
    pipeline (PASS A of piece p+1 overlaps PASS B of piece p):
      PASS A: K projection in transposed form (kT [128 hd, t] per kv head),
        RMSNorm across partitions (gpsimd partition_all_reduce), RoPE via a
        +-1 permutation matmul (rot = PT.T @ kn); V projection in natural
        form (v [t, 512] bf16).
      PASS B: scoresT [l, 4q*64] = kT-tile.T @ qT4 (fp32r), + maskT,
        exp -> bf16; a 1-row ones-matmul + DVE add accumulates sumexp in
        SBUF; the v-matmul accumulates unnormalized attention output in
        PSUM [128 hd, 1024] across all pieces (start= only on the first
        matmul per PSUM bank - start clears has_written bank-wide).
  - Noise piece (the 64 hidden_states tokens) runs the same A/B pipeline.
  - Epilogue: recip(sumexp), gpsimd partition_broadcast, normalize to bf16,
    o-projection against prefetched bf16 WoT, DMA out [64, 4096].

Softmax skips max-subtraction: scores = q.k/sqrt(128) + mask are bounded
(|q|,|k| <= sqrt(128) after RMSNorm => |score| <= ~16), so exp stays well
inside fp32 range and the result is mathematically identical.
"""

import os
from contextlib import ExitStack

import ml_dtypes
import numpy as np

import concourse.bass as bass
import concourse.bass_isa as bass_isa
import concourse.mybir as mybir
import concourse.tile as tile
from concourse import bacc
from concourse.bass_utils import run_bass_kernel_spmd

F32 = mybir.dt.float32
F32R = mybir.dt.float32r
BF16 = mybir.dt.bfloat16
AF = mybir.ActivationFunctionType
OP = mybir.AluOpType

H = 4096
NH = 32
NKV = 8
HD = 128
KQ = 64          # number of query tokens
NKVL = 4         # kv heads per core
NQL = 16         # q heads per core
DKV = NKVL * HD  # 512
DQ = NQL * HD    # 2048
PIECE = 256      # context tokens per streamed piece
EPS = 1e-6


def build_program(n_pieces=16, debug_taps=False):
    """Build the per-core Bass program. ctx = n_pieces * PIECE tokens."""
    ctx_len = n_pieces * PIECE
    L = ctx_len + KQ
    nht = H // 128  # 32 h-tiles

    nc = bacc.Bacc("TRN2", target_bir_lowering=False, debug=False, num_devices=8)

    # ---- DRAM parameters (per-core shards, host-prepared layouts) ----
    xT_d = nc.dram_tensor("xT", [H, ctx_len], BF16, kind="ExternalInput").ap()
    xnT_d = nc.dram_tensor("xnT", [H, KQ], BF16, kind="ExternalInput").ap()
    wkT_d = nc.dram_tensor("wkT", [H, DKV], BF16, kind="ExternalInput").ap()
    wvT_d = nc.dram_tensor("wvT", [H, DKV], BF16, kind="ExternalInput").ap()
    wqT_d = nc.dram_tensor("wqT", [H, DQ], BF16, kind="ExternalInput").ap()
    woT_d = nc.dram_tensor("woT", [DQ, H], BF16, kind="ExternalInput").ap()
    cosT_d = nc.dram_tensor("cosT", [HD, L], F32, kind="ExternalInput").ap()
    sinT_d = nc.dram_tensor("sinT", [HD, L], F32, kind="ExternalInput").ap()
    cosq_d = nc.dram_tensor("cosq", [KQ, HD], F32, kind="ExternalInput").ap()
    sinq_d = nc.dram_tensor("sinq", [KQ, HD], F32, kind="ExternalInput").ap()
    maskT_d = nc.dram_tensor("maskT", [L, KQ], F32, kind="ExternalInput").ap()
    qw_d = nc.dram_tensor("qw", [KQ, HD], F32, kind="ExternalInput").ap()
    kw_d = nc.dram_tensor("kw", [HD, 1], F32, kind="ExternalInput").ap()
    pt_d = nc.dram_tensor("pt", [HD, HD], F32R, kind="ExternalInput").ap()
    id64_d = nc.dram_tensor("id64", [KQ, KQ], F32, kind="ExternalInput").ap()
    out_d = nc.dram_tensor("out", [KQ, H], F32, kind="ExternalOutput").ap()
    if debug_taps:
        dbg_q_d = nc.dram_tensor("dbg_q", [KQ, DQ], F32, kind="ExternalOutput").ap()
        dbg_qr_d = nc.dram_tensor("dbg_qr", [KQ, DQ], F32, kind="ExternalOutput").ap()
        dbg_qT_d = nc.dram_tensor("dbg_qT", [HD, 256], F32, kind="ExternalOutput").ap()
        dbg_kT_d = nc.dram_tensor("dbg_kT", [HD, PIECE], F32, kind="ExternalOutput").ap()
        dbg_v_d = nc.dram_tensor("dbg_v", [128, DKV], F32, kind="ExternalOutput").ap()
        dbg_scm_d = nc.dram_tensor("dbg_scm", [128, 256], F32, kind="ExternalOutput").ap()
        dbg_sums_d = nc.dram_tensor("dbg_sums", [1, 1024], F32, kind="ExternalOutput").ap()
        dbg_attn_d = nc.dram_tensor("dbg_attn", [128, 1024], F32, kind="ExternalOutput").ap()

    xT_r = xT_d.rearrange("(ht p) t -> p ht t", p=128)
    xnT_r = xnT_d.rearrange("(ht p) t -> p ht t", p=128)
    wkT_r = wkT_d.rearrange("(ht p) d -> p ht d", p=128)
    wvT_r = wvT_d.rearrange("(ht p) d -> p ht d", p=128)
    wqT_r = wqT_d.rearrange("(ht p) d -> p ht d", p=128)
    woT_r = woT_d.rearrange("(ot p) h -> p ot h", p=128)
    maskT_c = maskT_d[0:ctx_len].rearrange("(lt p) q -> p lt q", p=128)

    with tile.TileContext(nc) as tc, ExitStack() as ctx:
        consts = ctx.enter_context(tc.tile_pool(name="consts", bufs=1))
        accps = ctx.enter_context(tc.tile_pool(name="accps", bufs=1, space="PSUM"))

        # ---- persistent PSUM accumulator (2 banks) ----
        o_ps = accps.tile([128, NKVL * 256], mybir.dt.float32)

        # ---- small resident constants ----
        cosq_sb = consts.tile([KQ, HD], F32)
        sinq_sb = consts.tile([KQ, HD], F32)
        qw_sb = consts.tile([KQ, HD], F32)
        kw_sb = consts.tile([HD, 1], F32)
        pt_sb = consts.tile([HD, HD], F32R)
        id64_sb = consts.tile([KQ, KQ], F32)
        ones_sb = consts.tile([128, 1], BF16)
        nc.vector.memset(ones_sb, 1.0)
        eps_sb = consts.tile([128, 1], F32)
        nc.vector.memset(eps_sb, EPS)
        sums_sb = consts.tile([1, NKVL * 256], F32)
        nc.vector.memset(sums_sb, 0.0)
        xn_sb = consts.tile([128, nht, KQ], BF16)
        qT4 = [consts.tile([HD, 256], F32R, name=f"qT4_{kh}", tag=f"qT4_{kh}")
               for kh in range(NKVL)]
        q_sb = consts.tile([KQ, NQL, HD], F32)


        nc.sync.dma_start(out=xn_sb, in_=xnT_r)
        nc.sync.dma_start(out=cosq_sb, in_=cosq_d)
        nc.sync.dma_start(out=sinq_sb, in_=sinq_d)
        nc.sync.dma_start(out=qw_sb, in_=qw_d)
        nc.sync.dma_start(out=kw_sb, in_=kw_d)
        nc.sync.dma_start(out=pt_sb, in_=pt_d)
        nc.sync.dma_start(out=id64_sb, in_=id64_d)

        with tc.tile_pool(name="csp", bufs=2) as csp, \
             tc.tile_pool(name="vtmp", bufs=6) as vtmp, \
             tc.tile_pool(name="ktmp", bufs=2) as ktmp, \
             tc.tile_pool(name="kTp", bufs=10) as kTp, \
             tc.tile_pool(name="kps", bufs=2, space="PSUM") as kps, \
             tc.tile_pool(name="scp", bufs=3, space="PSUM") as scp, \
             tc.tile_pool(name="vps", bufs=1, space="PSUM") as vps:
            wkv_ctx = ExitStack()
            wkv = wkv_ctx.enter_context(tc.tile_pool(name="wkv", bufs=1))
            wk_sb = wkv.tile([128, nht, DKV], BF16)
            wv_sb = wkv.tile([128, nht, DKV], BF16)
            xp_ctx = ExitStack()
            xp = xp_ctx.enter_context(tc.tile_pool(name="xp", bufs=2))
            qph_ctx = ExitStack()
            qph = qph_ctx.enter_context(tc.tile_pool(name="qph", bufs=1))
            qwp_ctx = ExitStack()
            qwp = qwp_ctx.enter_context(tc.tile_pool(name="qwp", bufs=2))
            wop_ctx = ExitStack()

            pstate = {}

            def emit_A(p):
                last_piece = p == n_pieces
                if not last_piece:
                    tlen = PIECE
                    x_sb = xp.tile([128, nht, PIECE], BF16, name=f"x_{p}", tag="x")
                    nc.sync.dma_start(out=x_sb, in_=xT_r[:, :, p * PIECE:(p + 1) * PIECE])
                    ltiles = [(0, 128), (128, 128)]
                    cos_off = p * PIECE
                else:
                    tlen = KQ
                    x_sb = xn_sb
                    ltiles = [(0, KQ)]
                    cos_off = ctx_len
                cos_sl = csp.tile([HD, tlen], F32, name=f"cos_{p}", tag="cos")
                sin_sl = csp.tile([HD, tlen], F32, name=f"sin_{p}", tag="sin")
                nc.sync.dma_start(out=cos_sl, in_=cosT_d[:, cos_off:cos_off + tlen])
                nc.sync.dma_start(out=sin_sl, in_=sinT_d[:, cos_off:cos_off + tlen])
                if not last_piece:
                    msk_sl = csp.tile([128, 2, KQ], F32, name=f"msk_{p}", tag="msk")
                    nc.sync.dma_start(out=msk_sl, in_=maskT_c[:, 2 * p:2 * p + 2, :])
                else:
                    msk_sl = csp.tile([KQ, KQ], F32, name="msk_n", tag="mskn")
                    nc.sync.dma_start(out=msk_sl, in_=maskT_d[ctx_len:L])

                # ---- V projection (natural layout), per l-tile ----
                v_sbs = []
                for lt, (lo, lsz) in enumerate(ltiles):
                    v_ps = vps.tile([lsz, DKV], mybir.dt.float32,
                                    name=f"v_ps_{p}_{lt}", tag="vp")
                    for ht in range(nht):
                        nc.tensor.matmul(v_ps, x_sb[:, ht, lo:lo + lsz], wv_sb[:, ht, :],
                                         start=(ht == 0), stop=(ht == nht - 1))
                    v_sb = vtmp.tile([lsz, DKV], BF16, name=f"v_sb_{p}_{lt}", tag="v")
                    nc.vector.tensor_copy(v_sb, v_ps)
                    v_sbs.append(v_sb)
                    if debug_taps and p == 0 and lt == 0:
                        dbgv = vtmp.tile([128, DKV], F32, tag="dbgv")
                        nc.vector.tensor_copy(dbgv, v_ps)
                        nc.sync.dma_start(out=dbg_v_d, in_=dbgv)

                # ---- K projection + RMSNorm + RoPE for all kv heads ----
                kTs = []
                for kh in range(NKVL):
                    k_ps = kps.tile([HD, tlen], mybir.dt.float32,
                                    name=f"k_ps_{p}_{kh}", tag="kp")
                    for ht in range(nht):
                        nc.tensor.matmul(k_ps, wk_sb[:, ht, kh * HD:(kh + 1) * HD],
                                         x_sb[:, ht, :],
                                         start=(ht == 0), stop=(ht == nht - 1))
                    kraw = ktmp.tile([HD, tlen], F32, name=f"kraw_{p}_{kh}", tag="kraw")
                    nc.vector.tensor_copy(kraw, k_ps)
                    k2 = ktmp.tile([HD, tlen], F32, name=f"k2_{p}_{kh}", tag="k2")
                    nc.vector.tensor_mul(k2, kraw, kraw)
                    s_t = ktmp.tile([HD, tlen], F32, name=f"s_{p}_{kh}", tag="s")
                    nc.gpsimd.partition_all_reduce(s_t, k2, channels=128,
                                                   reduce_op=bass_isa.ReduceOp.add)
                    nc.scalar.activation(s_t, s_t, AF.Sqrt, bias=eps_sb, scale=1.0 / HD)
                    r_t = ktmp.tile([HD, tlen], F32, name=f"r_{p}_{kh}", tag="r")
                    nc.vector.reciprocal(r_t, s_t)
                    kn = ktmp.tile([HD, tlen], F32R, name=f"kn_{p}_{kh}", tag="kn")
                    nc.vector.scalar_tensor_tensor(kn, kraw, kw_sb, r_t,
                                                   op0=OP.mult, op1=OP.mult)
                    # RoPE: kT = kn*cos + (PT.T @ kn)*sin
                    rot_ps = scp.tile([HD, tlen], mybir.dt.float32,
                                      name=f"rot_{p}_{kh}", tag="pp")
                    nc.tensor.matmul(rot_ps, pt_sb, kn, start=True, stop=True)
                    kT = kTp.tile([HD, tlen], F32R, name=f"kT_{p}_{kh}", tag="kT")
                    nc.vector.tensor_mul(kT, kn, cos_sl)
                    nc.vector.tensor_mul(k2, rot_ps, sin_sl)
                    nc.vector.tensor_add(kT, kT, k2)
                    kTs.append(kT)
                    if debug_taps and p == 0 and kh == 0:
                        dbgkT = ktmp.tile([HD, PIECE], F32, tag="dbgkT")
                        nc.vector.tensor_copy(dbgkT, kT)
                        nc.sync.dma_start(out=dbg_kT_d, in_=dbgkT)

                pstate[p] = (ltiles, v_sbs, kTs, msk_sl)

            def emit_B(p):
                last_piece = p == n_pieces
                ltiles, v_sbs, kTs, msk_sl = pstate.pop(p)
                if p == 0:
                    # q transposes: the q DVE chain completed during PASS A of
                    # pieces 0/1, so these PE ops do not stall.
                    for kh in range(NKVL):
                        for qh in range(4):
                            t_ps = scp.tile([HD, KQ], mybir.dt.float32,
                                            name=f"tq_{kh}_{qh}", tag="pp")
                            nc.tensor.transpose(t_ps, q_sb[:, kh * 4 + qh, :], id64_sb)
                            nc.vector.tensor_copy(qT4[kh][:, qh * 64:(qh + 1) * 64], t_ps)
                    if debug_taps:
                        dbgqT = ktmp.tile([HD, 256], F32, tag="dbgqT")
                        nc.vector.tensor_copy(dbgqT, qT4[0])
                        nc.sync.dma_start(out=dbg_qT_d, in_=dbgqT)
                # phase 1: all score matmuls + mask-add + exp (the DVE/ACT
                # chain for head i pipelines under head i+1's score matmul)
                expTs = {}
                for kh in range(NKVL):
                    kT = kTs[kh]
                    for lt, (lo, lsz) in enumerate(ltiles):
                        sc_ps = scp.tile([lsz, 256], mybir.dt.float32,
                                         name=f"sc_{p}_{kh}_{lt}", tag="pp")
                        nc.tensor.matmul(sc_ps, kT[:, lo:lo + lsz],
                                         qT4[kh], start=True, stop=True)
                        scm = ktmp.tile([lsz, 4, KQ], F32,
                                        name=f"scm_{p}_{kh}_{lt}", tag="scm", bufs=3)
                        if not last_piece:
                            msk = msk_sl[0:lsz, lt, :]
                        else:
                            msk = msk_sl
                        nc.vector.tensor_add(scm, sc_ps.rearrange("l (g q) -> l g q", g=4),
                                             msk.unsqueeze(1).to_broadcast((lsz, 4, KQ)))
                        if debug_taps and p == 0 and kh == 0 and lt == 0:
                            dbgscm = ktmp.tile([lsz, 256], F32, tag="dbgscm")
                            nc.vector.tensor_copy(dbgscm.rearrange("l (g q) -> l g q", g=4), scm)
                            nc.sync.dma_start(out=dbg_scm_d[0:lsz, :], in_=dbgscm)
                        expT = ktmp.tile([lsz, 256], BF16,
                                         name=f"expT_{p}_{kh}_{lt}", tag="expT", bufs=10)
                        nc.scalar.activation(expT.rearrange("l (g q) -> l g q", g=4), scm, AF.Exp)
                        expTs[(kh, lt)] = expT
                # phase 2: sumexp + output accumulation matmuls
                for kh in range(NKVL):
                    for lt, (lo, lsz) in enumerate(ltiles):
                        first = p == 0 and lt == 0
                        # start=True clears has_written for the WHOLE bank, so
                        # only the first matmul touching each o_ps bank may set
                        # it (kh 0/1 share a bank; kh 2/3 share a bank).
                        first_bank = first and kh % 2 == 0
                        last = last_piece
                        expT = expTs[(kh, lt)]
                        summ_ps = scp.tile([1, 256], mybir.dt.float32,
                                           name=f"sm_{p}_{kh}_{lt}", tag="pp")
                        nc.tensor.matmul(summ_ps, ones_sb[0:lsz, :], expT,
                                         start=True, stop=True)
                        nc.vector.tensor_add(sums_sb[:, kh * 256:(kh + 1) * 256],
                                             sums_sb[:, kh * 256:(kh + 1) * 256], summ_ps)
                        nc.tensor.matmul(o_ps[:, kh * 256:(kh + 1) * 256],
                                         v_sbs[lt][:, kh * HD:(kh + 1) * HD], expT,
                                         start=first_bank, stop=last, skip_group_check=True)

            # ---- q projection: 16 weight slices, DMAs feeding just-in-time;
            # the first half streams before piece-0 data, the second half after.
            def emit_q_half(half):
                for ds in range(half * 8, half * 8 + 8):
                    wq_sb = qwp.tile([128, nht, 128], BF16, name=f"wq_{ds}", tag="wq")
                    nc.sync.dma_start(out=wq_sb, in_=wqT_r[:, :, ds * 128:(ds + 1) * 128])
                    if ds == 1:
                        nc.sync.dma_start(out=wv_sb[:, 0:16, :], in_=wvT_r[:, 0:16, :])
                    elif ds == 3:
                        nc.sync.dma_start(out=wv_sb[:, 16:32, :], in_=wvT_r[:, 16:32, :])
                    elif ds == 5:
                        nc.sync.dma_start(out=wk_sb[:, 0:16, :], in_=wkT_r[:, 0:16, :])
                    elif ds == 7:
                        nc.sync.dma_start(out=wk_sb[:, 16:32, :], in_=wkT_r[:, 16:32, :])
                    q_ps = kps.tile([KQ, 128], mybir.dt.float32,
                                    name=f"q_ps_{ds}", tag="kp")
                    for ht in range(nht):
                        nc.tensor.matmul(q_ps, xn_sb[:, ht, :], wq_sb[:, ht, :],
                                         start=(ht == 0), stop=(ht == nht - 1))
                    nc.vector.tensor_copy(
                        q_sb.rearrange("q nh hd -> q (nh hd)")[:, ds * 128:(ds + 1) * 128],
                        q_ps)

            emit_q_half(0)
            emit_A(0)
            emit_q_half(1)

            # ---- q RMSNorm + RoPE (DVE chain overlaps piece-0/1 matmuls) ----
            q2 = qph.tile([KQ, NQL, HD], F32, tag="qbig", bufs=2)
            nc.vector.tensor_mul(q2, q_sb, q_sb)
            ss = qph.tile([KQ, NQL], F32, tag="ss")
            nc.vector.reduce_sum(ss, q2, axis=mybir.AxisListType.X)
            sq = qph.tile([KQ, NQL], F32, tag="sq")
            nc.scalar.activation(sq, ss, AF.Sqrt, bias=eps_sb[0:KQ], scale=1.0 / HD)
            rq = qph.tile([KQ, NQL], F32, tag="rq")
            nc.vector.reciprocal(rq, sq)
            qn = qph.tile([KQ, NQL, HD], F32, tag="qbig", bufs=2)
            nc.vector.tensor_mul(qn, q_sb, rq.unsqueeze(2).to_broadcast((KQ, NQL, HD)))
            nc.vector.tensor_mul(qn, qn, qw_sb.unsqueeze(1).to_broadcast((KQ, NQL, HD)))
            rot = qph.tile([KQ, NQL, HD], F32, tag="qbig", bufs=2)
            nc.vector.tensor_scalar_mul(rot[:, :, 0:64], qn[:, :, 64:128], -1.0)
            nc.vector.tensor_copy(rot[:, :, 64:128], qn[:, :, 0:64])
            if debug_taps:
                nc.sync.dma_start(out=dbg_q_d, in_=q_sb.rearrange("q nh hd -> q (nh hd)"))
            nc.vector.tensor_mul(q_sb, qn, cosq_sb.unsqueeze(1).to_broadcast((KQ, NQL, HD)))
            nc.vector.tensor_mul(rot, rot, sinq_sb.unsqueeze(1).to_broadcast((KQ, NQL, HD)))
            nc.vector.tensor_add(q_sb, q_sb, rot)
            if debug_taps:
                nc.sync.dma_start(out=dbg_qr_d, in_=q_sb.rearrange("q nh hd -> q (nh hd)"))
            qwp_ctx.close()
            qph_ctx.close()

            emit_A(1)
            for p in range(n_pieces + 1):
                emit_B(p)
                if p + 2 <= n_pieces:
                    emit_A(p + 2)
                if p == n_pieces - 2:
                    # the last x tile and the kv weights are consumed by
                    # A(15)/A(16): free both pools and prefetch the
                    # o-projection weights during the remaining pieces
                    xp_ctx.close()
                    wkv_ctx.close()
                    wop = wop_ctx.enter_context(tc.tile_pool(name="wop", bufs=6))
                    wo_tiles = []
                    for ho in range(H // 512):
                        wo_sb = wop.tile([128, NQL, 512], BF16, name=f"wo_{ho}", tag="wo")
                        nc.sync.dma_start(out=wo_sb, in_=woT_r[:, :, ho * 512:(ho + 1) * 512])
                        wo_tiles.append(wo_sb)

            # ============ epilogue: normalize + o-projection ============
            ep_ctx = ExitStack()
            ep = ep_ctx.enter_context(tc.tile_pool(name="ep", bufs=1))
            eps2 = ep_ctx.enter_context(tc.tile_pool(name="eps2", bufs=2))
            rec_sb = ep.tile([1, NKVL * 256], F32)
            nc.vector.reciprocal(rec_sb, sums_sb)
            rec_bc = ep.tile([128, NKVL * 256], F32)
            nc.gpsimd.partition_broadcast(rec_bc, rec_sb, channels=128)
            attn_sb = ep.tile([128, NKVL * 256], BF16)
            nc.vector.tensor_mul(attn_sb, o_ps, rec_bc)
            if debug_taps:
                nc.sync.dma_start(out=dbg_sums_d, in_=sums_sb)
                dbga = ep.tile([128, 1024], F32)
                nc.vector.tensor_mul(dbga, o_ps, rec_bc)
                nc.sync.dma_start(out=dbg_attn_d, in_=dbga)

            for ho in range(H // 512):
                out_ps = scp.tile([KQ, 512], mybir.dt.float32, name=f"op_{ho}", tag="pp")
                for ot in range(NQL):
                    nc.tensor.matmul(out_ps, attn_sb[:, ot * 64:(ot + 1) * 64],
                                     wo_tiles[ho][:, ot, :],
                                     start=(ot == 0), stop=(ot == NQL - 1))
                out_sb = eps2.tile([KQ, 512], F32, name=f"ob_{ho}", tag="ob")
                nc.vector.tensor_copy(out_sb, out_ps)
                nc.sync.dma_start(out=out_d[:, ho * 512:(ho + 1) * 512], in_=out_sb)
            ep_ctx.close()
            wop_ctx.close()

    nc.compile()
    return nc


_prog_cache = {}


def _get_program(n_pieces, debug_taps=False):
    key = (n_pieces, debug_taps)
    if key not in _prog_cache:
        _prog_cache[key] = build_program(n_pieces, debug_taps)
    return _prog_cache[key]


def make_in_maps(hidden_states, target_hidden, attn_mask, cos, sin,
                 Wq, Wk, Wv, Wo, q_norm_w, k_norm_w):
    """Host-side sharding/layout prep -> 8 per-core input maps."""
    B, K, _ = hidden_states.shape
    bf = ml_dtypes.bfloat16

    # rotate-half permutation (as lhsT): rot = P @ k, pass PT = P.T
    P = np.zeros((HD, HD), np.float32)
    for i in range(HD // 2):
        P[i, i + HD // 2] = -1.0
        P[i + HD // 2, i] = 1.0
    PT = np.ascontiguousarray(P.T)
    id64 = np.eye(KQ, dtype=np.float32)

    qw = (np.broadcast_to(q_norm_w, (KQ, HD)) / np.sqrt(HD)).astype(np.float32)
    qw = np.ascontiguousarray(qw)
    kw = np.ascontiguousarray(k_norm_w.reshape(HD, 1).astype(np.float32))

    in_maps = []
    for core in range(8):
        b, g = divmod(core, 2)
        xT = np.ascontiguousarray(target_hidden[b].T).astype(bf)
        xnT = np.ascontiguousarray(hidden_states[b].T).astype(bf)
        wkT = np.ascontiguousarray(Wk[g * DKV:(g + 1) * DKV].T).astype(bf)
        wvT = np.ascontiguousarray(Wv[g * DKV:(g + 1) * DKV].T).astype(bf)
        wqT = np.ascontiguousarray(Wq[g * DQ:(g + 1) * DQ].T).astype(bf)
        woT = np.ascontiguousarray(Wo[:, g * DQ:(g + 1) * DQ].T).astype(bf)
        cosT = np.ascontiguousarray(cos[b].T).astype(np.float32)
        sinT = np.ascontiguousarray(sin[b].T).astype(np.float32)
        cosq = np.ascontiguousarray(cos[b, -K:, :]).astype(np.float32)
        sinq = np.ascontiguousarray(sin[b, -K:, :]).astype(np.float32)
        maskT = np.ascontiguousarray(attn_mask[b, 0].T).astype(np.float32)
        in_maps.append({
            "xT": xT, "xnT": xnT, "wkT": wkT, "wvT": wvT, "wqT": wqT,
            "woT": woT, "cosT": cosT, "sinT": sinT, "cosq": cosq,
            "sinq": sinq, "maskT": maskT, "qw": qw, "kw": kw,
            "pt": PT, "id64": id64,
        })
    return in_maps


def kernel(hidden_states, target_hidden, attn_mask, cos, sin,
           Wq, Wk, Wv, Wo, q_norm_w, k_norm_w):
    B, K, _ = hidden_states.shape
    ctx_len = target_hidden.shape[1]
    assert ctx_len % PIECE == 0
    n_pieces = ctx_len // PIECE
    nc = _get_program(n_pieces,
                      debug_taps=os.environ.get("KERNEL_DEBUG_TAPS", "0") == "1")
    in_maps = make_in_maps(hidden_states, target_hidden, attn_mask, cos, sin,
                           Wq, Wk, Wv, Wo, q_norm_w, k_norm_w)
    res = run_bass_kernel_spmd(nc, in_maps, core_ids=list(range(8)),
                               trace=os.environ.get("KERNEL_TRACE", "0") == "1")
    out = np.zeros((B, K, H), np.float32)
    for core in range(8):
        b = core // 2
        out[b] += res.results[core]["out"]
    kernel.last_results = res
    return out



# revision 14
# speedup vs baseline: 1.1302x; 1.1302x over previous
"""DFlashAttention kernel for Trainium2, 8 NeuronCores.

Sharding: 8 cores = 4 batches x 2 KV-head-groups. Each core (b, g) handles
batch b and KV heads [4g, 4g+4) (query heads [16g, 16g+16)), producing the
partial output  sum_{o in group} attn[:, o] @ WoT[o, :]  for its batch. The
host sums the two group partials per batch (row-parallel o-projection).

v2 redesign (vs v1) driven by TimelineSim engine-occupancy analysis:
  - PE engine was 84% busy; PE row-count floor ~524 us.  All changes either
    cut PE rows or close PE idle gaps (which also avoid p-state resets).
  - Q projection emitted TRANSPOSED (out [hd, tok] per head, 64-row
    matmuls): half the q-proj PE rows, no PE transposes, no big DVE q
    chain; q RMSNorm/RoPE runs per 4-head group exactly like the K path.
  - sumexp moved off PE (was 132 ones-matmuls = 14.5 us) to gpsimd
    partition_all_reduce on the Pool engine (4.5% busy), accumulated into
    a [1, 1024] SBUF row, one partition_broadcast at the end.
  - K RMSNorm reads k_ps directly from PSUM (no kraw copy) and uses ACT
    Rsqrt (no DVE reciprocal).
  - rot (RoPE permutation) matmuls emitted AFTER the next head's K
    projection so the PE never stalls on the DVE norm chain (was ~4
    stalls x ~0.9 us per piece).
  - B-pass emission split: scores before A(p+2).K, attn-accumulate after
    A(p+2).V, so exp/DVE latency is covered by ~20 us of PE work.
  - DMA: weight loads use >=512B contiguous elements (wq was paying the
    <512B 2x descriptor penalty: 93 us -> 47 us), cos/sin packed into one
    [128, 2, L] tensor (one DMA per piece), mask preloaded in 2 DMAs,
    startup order wk -> x0 -> cs0 -> wv -> (wq chunk + q-proj + B0.kh)*4
    so the PE starts ~9 us in and B(0) streams per-head.

Softmax skips max-subtraction: scores = q.k/sqrt(128) + mask are bounded
(|q|,|k| <= sqrt(128) after RMSNorm => |score| <= ~16), so exp stays well
inside fp32 range and the result is mathematically identical.
"""

import os
from contextlib import ExitStack

import ml_dtypes
import numpy as np

import concourse.bass as bass
import concourse.bass_isa as bass_isa
import concourse.mybir as mybir
import concourse.tile as tile
from concourse import bacc
from concourse.bass_utils import run_bass_kernel_spmd

F32 = mybir.dt.float32
F32R = mybir.dt.float32r
BF16 = mybir.dt.bfloat16
AF = mybir.ActivationFunctionType
OP = mybir.AluOpType
RED = bass_isa.ReduceOp

H = 4096
NH = 32
NKV = 8
HD = 128
KQ = 64          # number of query tokens
NKVL = 4         # kv heads per core
NQL = 16         # q heads per core
DKV = NKVL * HD  # 512
DQ = NQL * HD    # 2048
PIECE = 256      # context tokens per streamed piece
EPS = 1e-6


def build_program(n_pieces=16):
    """Build the per-core Bass program. ctx = n_pieces * PIECE tokens."""
    ctx_len = n_pieces * PIECE
    L = ctx_len + KQ
    nlt = (L + 127) // 128          # 33 mask l-tiles (host pads to nlt*128)
    nht = H // 128                  # 32 h-tiles

    nc = bacc.Bacc("TRN2", target_bir_lowering=False, debug=False, num_devices=8)

    # ---- DRAM parameters (per-core shards, host-prepared layouts) ----
    xT_d = nc.dram_tensor("xT", [H, ctx_len], BF16, kind="ExternalInput").ap()
    xnT_d = nc.dram_tensor("xnT", [H, KQ], BF16, kind="ExternalInput").ap()
    wkT_d = nc.dram_tensor("wkT", [H, DKV], BF16, kind="ExternalInput").ap()
    wvT_d = nc.dram_tensor("wvT", [H, DKV], BF16, kind="ExternalInput").ap()
    wqT_d = nc.dram_tensor("wqT", [H, DQ], BF16, kind="ExternalInput").ap()
    woT_d = nc.dram_tensor("woT", [DQ, H], BF16, kind="ExternalInput").ap()
    csT_d = nc.dram_tensor("csT", [HD, 2, L], F32, kind="ExternalInput").ap()
    maskT_d = nc.dram_tensor("maskT", [nlt * 128, KQ], F32, kind="ExternalInput").ap()
    qw_d = nc.dram_tensor("qw", [HD, 1], F32, kind="ExternalInput").ap()
    kw_d = nc.dram_tensor("kw", [HD, 1], F32, kind="ExternalInput").ap()
    pt_d = nc.dram_tensor("pt", [HD, HD], F32R, kind="ExternalInput").ap()
    out_d = nc.dram_tensor("out", [KQ, H], F32, kind="ExternalOutput").ap()

    xT_r = xT_d.rearrange("(ht p) t -> p ht t", p=128)
    xnT_r = xnT_d.rearrange("(ht p) t -> p ht t", p=128)
    wkT_r = wkT_d.rearrange("(ht p) d -> p ht d", p=128)
    wvT_r = wvT_d.rearrange("(ht p) d -> p ht d", p=128)
    wqT_r = wqT_d.rearrange("(ht p) d -> p ht d", p=128)
    woT_r = woT_d.rearrange("(ot p) h -> p ot h", p=128)
    maskT_r = maskT_d.rearrange("(lt p) q -> p lt q", p=128)

    with tile.TileContext(nc) as tc, ExitStack() as ctx:
        consts = ctx.enter_context(tc.tile_pool(name="consts", bufs=1))
        accps = ctx.enter_context(tc.tile_pool(name="accps", bufs=1, space="PSUM"))

        # ---- persistent PSUM accumulator (2 banks) ----
        o_ps = accps.tile([128, NKVL * 256], mybir.dt.float32)

        # ---- small resident constants ----
        qw_sb = consts.tile([HD, 1], F32)
        kw_sb = consts.tile([HD, 1], F32)
        pt_sb = consts.tile([HD, HD], F32R)
        eps_sb = consts.tile([128, 1], F32)
        nc.vector.memset(eps_sb, EPS)
        sums_sb = consts.tile([1, NKVL * 256], F32)
        nc.vector.memset(sums_sb, 0.0)
        csq_sb = consts.tile([128, 2, KQ], F32)
        xn_sb = consts.tile([128, nht, KQ], BF16)
        mask_sb = consts.tile([128, nlt, KQ], F32)
        qT4 = [consts.tile([HD, 256], F32R, name=f"qT4_{kh}", tag=f"qT4_{kh}")
               for kh in range(NKVL)]

        # tiny first, then what the PE needs soonest
        nc.sync.dma_start(out=qw_sb, in_=qw_d)
        nc.sync.dma_start(out=kw_sb, in_=kw_d)
        nc.sync.dma_start(out=pt_sb, in_=pt_d)
        nc.sync.dma_start(out=csq_sb, in_=csT_d[:, :, ctx_len:L])

        with tc.tile_pool(name="csp", bufs=3) as csp, \
             tc.tile_pool(name="vtmp", bufs=6) as vtmp, \
             tc.tile_pool(name="ktmp", bufs=2) as ktmp, \
             tc.tile_pool(name="kTp", bufs=12) as kTp, \
             tc.tile_pool(name="kps", bufs=2, space="PSUM") as kps, \
             tc.tile_pool(name="scp", bufs=3, space="PSUM") as scp, \
             tc.tile_pool(name="vps", bufs=1, space="PSUM") as vps:
            wkv_ctx = ExitStack()
            wkv = wkv_ctx.enter_context(tc.tile_pool(name="wkv", bufs=1))
            wk_sb = wkv.tile([128, nht, DKV], BF16)
            wv_sb = wkv.tile([128, nht, DKV], BF16)
            xp_ctx = ExitStack()
            xp = xp_ctx.enter_context(tc.tile_pool(name="xp", bufs=2))
            qwp_ctx = ExitStack()
            qwp = qwp_ctx.enter_context(tc.tile_pool(name="qwp", bufs=1))
            wop_ctx = ExitStack()

            pstate = {}

            def emit_A_dma(p):
                """DMAs for piece p (x already handled for p=0 specially)."""
                if p > 0 and p < n_pieces:
                    x_sb = xp.tile([128, nht, PIECE], BF16, name=f"x_{p}", tag="x")
                    nc.sync.dma_start(out=x_sb, in_=xT_r[:, :, p * PIECE:(p + 1) * PIECE])
                    pstate[("x", p)] = x_sb
                if p < n_pieces:
                    cs_sl = csp.tile([128, 2, PIECE], F32, name=f"cs_{p}", tag="cs")
                    nc.sync.dma_start(out=cs_sl, in_=csT_d[:, :, p * PIECE:(p + 1) * PIECE])
                    pstate[("cs", p)] = cs_sl

            def emit_A_K(p):
                """K projection + RMSNorm for all kv heads of piece p (no rot)."""
                last_piece = p == n_pieces
                if last_piece:
                    tlen = KQ
                    x_sb = xn_sb
                    cs_sl = csq_sb
                else:
                    tlen = PIECE
                    x_sb = pstate.pop(("x", p))
                    cs_sl = pstate.pop(("cs", p))
                kps_l = []
                for kh in range(NKVL):
                    k_ps = kps.tile([HD, tlen], mybir.dt.float32,
                                    name=f"k_ps_{p}_{kh}", tag="kp")
                    for ht in range(nht):
                        nc.tensor.matmul(k_ps, wk_sb[:, ht, kh * HD:(kh + 1) * HD],
                                         x_sb[:, ht, :],
                                         start=(ht == 0), stop=(ht == nht - 1))
                    # RMSNorm chain straight off PSUM
                    k2 = ktmp.tile([HD, tlen], F32, name=f"k2_{p}_{kh}", tag="k2",
                                   bufs=2)
                    nc.scalar.activation(k2, k_ps, AF.Square)
                    s_t = ktmp.tile([HD, tlen], F32, name=f"s_{p}_{kh}", tag="s",
                                    bufs=2)
                    nc.gpsimd.partition_all_reduce(s_t, k2, channels=128,
                                                   reduce_op=RED.add)
                    r_t = ktmp.tile([HD, tlen], F32, name=f"r_{p}_{kh}", tag="r",
                                    bufs=2)
                    nc.scalar.activation(r_t, s_t, AF.Sqrt, bias=eps_sb,
                                         scale=1.0 / HD)
                    nc.vector.reciprocal(r_t, r_t)
                    kn = ktmp.tile([HD, tlen], F32R, name=f"kn_{p}_{kh}", tag="kn",
                                   bufs=6)
                    nc.vector.scalar_tensor_tensor(kn, k_ps, kw_sb, r_t,
                                                   op0=OP.mult, op1=OP.mult)
                    kps_l.append(kn)
                pstate[("kn", p)] = (tlen, x_sb, cs_sl, kps_l)

            def emit_A_rot(p):
                """RoPE for piece p: rot matmul + cos/sin combine -> kT."""
                tlen, x_sb, cs_sl, kns = pstate.pop(("kn", p))
                kTs = []
                for kh in range(NKVL):
                    kn = kns[kh]
                    rot_ps = scp.tile([HD, tlen], mybir.dt.float32,
                                      name=f"rot_{p}_{kh}", tag="sc")
                    nc.tensor.matmul(rot_ps, pt_sb, kn, start=True, stop=True)
                    kT = kTp.tile([HD, tlen], F32R, name=f"kT_{p}_{kh}", tag="kT")
                    nc.vector.tensor_mul(kT, kn, cs_sl[:, 0, :])
                    k2r = ktmp.tile([HD, tlen], F32, name=f"k2r_{p}_{kh}", tag="k2",
                                    bufs=2)
                    nc.vector.tensor_mul(k2r, rot_ps, cs_sl[:, 1, :])
                    nc.vector.tensor_add(kT, kT, k2r)
                    kTs.append(kT)
                pstate[("kT", p)] = kTs
                pstate[("xcs", p)] = (x_sb, cs_sl)

            def emit_A_V(p):
                """V projection (natural layout) for piece p."""
                last_piece = p == n_pieces
                x_sb, _ = pstate.pop(("xcs", p))
                ltiles = [(0, KQ)] if last_piece else [(0, 128), (128, 128)]
                v_sbs = []
                for lt, (lo, lsz) in enumerate(ltiles):
                    v_ps = vps.tile([lsz, DKV], mybir.dt.float32,
                                    name=f"v_ps_{p}_{lt}", tag="vp")
                    for ht in range(nht):
                        nc.tensor.matmul(v_ps, x_sb[:, ht, lo:lo + lsz],
                                         wv_sb[:, ht, :],
                                         start=(ht == 0), stop=(ht == nht - 1))
                    v_sb = vtmp.tile([lsz, DKV], BF16, name=f"v_sb_{p}_{lt}", tag="v")
                    nc.vector.tensor_copy(v_sb, v_ps)
                    v_sbs.append(v_sb)
                pstate[("v", p)] = (ltiles, v_sbs)

            def emit_B_scores(p, kh_list=None):
                """scoresT matmuls + mask-add + exp + Pool sumexp for piece p."""
                last_piece = p == n_pieces
                kTs = pstate[("kT", p)]
                ltiles = [(0, KQ)] if last_piece else [(0, 128), (128, 128)]
                expTs = pstate.setdefault(("expT", p), {})
                for kh in (kh_list if kh_list is not None else range(NKVL)):
                    kT = kTs[kh]
                    for lt, (lo, lsz) in enumerate(ltiles):
                        sc_ps = scp.tile([lsz, 256], mybir.dt.float32,
                                         name=f"sc_{p}_{kh}_{lt}", tag="sc")
                        nc.tensor.matmul(sc_ps, kT[:, lo:lo + lsz],
                                         qT4[kh], start=True, stop=True)
                        scm = ktmp.tile([lsz, 4, KQ], F32,
                                        name=f"scm_{p}_{kh}_{lt}", tag="scm", bufs=3)
                        if last_piece:
                            msk = mask_sb[0:KQ, 2 * n_pieces, :]
                        else:
                            msk = mask_sb[0:lsz, 2 * p + lt, :]
                        nc.vector.tensor_add(scm,
                                             sc_ps.rearrange("l (g q) -> l g q", g=4),
                                             msk.unsqueeze(1).to_broadcast((lsz, 4, KQ)))
                        expT = ktmp.tile([lsz, 256], BF16,
                                         name=f"expT_{p}_{kh}_{lt}", tag="expT",
                                         bufs=10)
                        nc.scalar.activation(expT.rearrange("l (g q) -> l g q", g=4),
                                             scm, AF.Exp)
                        # sumexp on Pool (PE stays free); accumulate row 0
                        sred = ktmp.tile([lsz, 256], F32,
                                         name=f"sred_{p}_{kh}_{lt}", tag="sred",
                                         bufs=3)
                        nc.gpsimd.partition_all_reduce(sred, expT, channels=lsz,
                                                       reduce_op=RED.add)
                        nc.vector.tensor_add(sums_sb[:, kh * 256:(kh + 1) * 256],
                                             sums_sb[:, kh * 256:(kh + 1) * 256],
                                             sred[0:1, :])
                        expTs[(kh, lt)] = expT

            def emit_B_attn(p, kh_list=None):
                """output-accumulation matmuls for piece p into o_ps."""
                last_piece = p == n_pieces
                ltiles, v_sbs = pstate[("v", p)]
                expTs = pstate[("expT", p)]
                for kh in (kh_list if kh_list is not None else range(NKVL)):
                    for lt, (lo, lsz) in enumerate(ltiles):
                        first_bank = p == 0 and lt == 0 and kh % 2 == 0
                        expT = expTs.pop((kh, lt))
                        nc.tensor.matmul(o_ps[:, kh * 256:(kh + 1) * 256],
                                         v_sbs[lt][:, kh * HD:(kh + 1) * HD], expT,
                                         start=first_bank, stop=last_piece,
                                         skip_group_check=True)
                if (kh_list is None or kh_list[-1] == NKVL - 1):
                    pstate.pop(("v", p))
                    pstate.pop(("expT", p))
                    if (("kT", p)) in pstate:
                        pstate.pop(("kT", p))

            def emit_q_group(g):
                """Transposed q projection + RMSNorm + RoPE for kv group g.

                Produces qT4[g] = [128 hd, 4 q-heads x 64 tok] f32r directly;
                no PE transposes, 64-row matmuls.
                """
                wq_sb = qwp.tile([128, nht, 512], BF16, name=f"wq_{g}", tag="wq")
                nc.sync.dma_start(out=wq_sb, in_=wqT_r[:, :, g * 512:(g + 1) * 512])
                q_ps = kps.tile([HD, 4, KQ], mybir.dt.float32,
                                name=f"q_ps_{g}", tag="kp")
                for qh in range(4):
                    for ht in range(nht):
                        nc.tensor.matmul(q_ps[:, qh, :],
                                         wq_sb[:, ht, qh * HD:(qh + 1) * HD],
                                         xn_sb[:, ht, :],
                                         start=(ht == 0), stop=(ht == nht - 1),
                                         skip_group_check=True)
                q2 = ktmp.tile([HD, 4, KQ], F32, name=f"q2_{g}", tag="k2", bufs=2)
                nc.scalar.activation(q2, q_ps, AF.Square)
                s_t = ktmp.tile([HD, 4, KQ], F32, name=f"qs_{g}", tag="s", bufs=2)
                nc.gpsimd.partition_all_reduce(
                    s_t.rearrange("p g q -> p (g q)"),
                    q2.rearrange("p g q -> p (g q)"),
                    channels=128, reduce_op=RED.add)
                r_t = ktmp.tile([HD, 4, KQ], F32, name=f"qr_{g}", tag="r", bufs=2)
                nc.scalar.activation(r_t, s_t, AF.Sqrt, bias=eps_sb, scale=1.0 / HD)
                nc.vector.reciprocal(r_t, r_t)
                qn = ktmp.tile([HD, 4, KQ], F32R, name=f"qn_{g}", tag="kn", bufs=6)
                nc.vector.scalar_tensor_tensor(qn, q_ps, qw_sb, r_t,
                                               op0=OP.mult, op1=OP.mult)
                rot_ps = scp.tile([HD, 4 * KQ], mybir.dt.float32,
                                  name=f"qrot_{g}", tag="sc")
                nc.tensor.matmul(rot_ps, pt_sb,
                                 qn.rearrange("p g q -> p (g q)"),
                                 start=True, stop=True)
                qt = qT4[g].rearrange("p (g q) -> p g q", g=4)
                nc.vector.tensor_mul(
                    qt, qn, csq_sb[:, 0:1, :].to_broadcast((HD, 4, KQ)))
                k2r = ktmp.tile([HD, 4, KQ], F32, name=f"qk2_{g}", tag="k2", bufs=2)
                nc.vector.tensor_mul(
                    k2r, rot_ps.rearrange("p (g q) -> p g q", g=4),
                    csq_sb[:, 1:2, :].to_broadcast((HD, 4, KQ)))
                nc.vector.tensor_add(qt, qt, k2r)

            # ================= startup schedule =================
            # DMA order == transfer order (single DMA_ENGINES resource):
            # wk halves, x0 halves, cs0, wv halves, then per-group wq chunks.
            nc.sync.dma_start(out=wk_sb[:, 0:16, :], in_=wkT_r[:, 0:16, :])
            x0_sb = xp.tile([128, nht, PIECE], BF16, name="x_0", tag="x")
            nc.sync.dma_start(out=x0_sb[:, 0:16, :], in_=xT_r[:, 0:16, 0:PIECE])
            nc.sync.dma_start(out=wk_sb[:, 16:32, :], in_=wkT_r[:, 16:32, :])
            nc.sync.dma_start(out=x0_sb[:, 16:32, :], in_=xT_r[:, 16:32, 0:PIECE])
            cs0_sl = csp.tile([128, 2, PIECE], F32, name="cs_0", tag="cs")
            nc.sync.dma_start(out=cs0_sl, in_=csT_d[:, :, 0:PIECE])
            nc.sync.dma_start(out=xn_sb, in_=xnT_r)
            nc.sync.dma_start(out=wv_sb[:, 0:16, :], in_=wvT_r[:, 0:16, :])
            nc.sync.dma_start(out=wv_sb[:, 16:32, :], in_=wvT_r[:, 16:32, :])
            pstate[("x", 0)] = x0_sb
            pstate[("cs", 0)] = cs0_sl

            # piece 0 K path (starts as soon as wk h0 + x0 h0 land)
            emit_A_K(0)
            emit_A_rot(0)
            emit_A_V(0)

            # mask part 1 (l-tiles 0..16 cover pieces 0..7); part 2 later
            nc.sync.dma_start(out=mask_sb[:, 0:17, :], in_=maskT_r[:, 0:17, :])

            # q groups stream: wq chunk g -> q-proj g -> B0 scores/attn for kh=g
            emit_q_group(0)
            emit_A_dma(1)
            emit_A_K(1)
            emit_B_scores(0, [0])
            emit_A_rot(1)
            emit_A_V(1)
            emit_B_attn(0, [0])
            emit_q_group(1)
            emit_A_dma(2)
            emit_A_K(2)
            emit_B_scores(0, [1])
            emit_A_rot(2)
            emit_A_V(2)
            emit_B_attn(0, [1])
            emit_q_group(2)
            emit_B_scores(0, [2])
            emit_B_attn(0, [2])
            emit_q_group(3)
            emit_B_scores(0, [3])
            emit_B_attn(0, [3])
            nc.sync.dma_start(out=mask_sb[:, 17:nlt, :], in_=maskT_r[:, 17:nlt, :])
            qwp_ctx.close()

            # ================= steady-state pipeline =================
            # per iteration p: B(p) scores -> A(p+2) K/rot -> B(p) attn ->
            # A(p+2) V.  exp/DVE of B(p) hides under A(p+2)'s ~20us PE work.
            for p in range(1, n_pieces + 1):
                emit_B_scores(p)
                if p + 2 <= n_pieces:
                    emit_A_dma(p + 2)
                    emit_A_K(p + 2)
                    emit_A_rot(p + 2)
                emit_B_attn(p)
                if p + 2 <= n_pieces:
                    emit_A_V(p + 2)
                if p == n_pieces - 2:
                    # x stream and kv weights fully consumed after A(16):
                    # free both pools and prefetch o-projection weights.
                    xp_ctx.close()
                    wkv_ctx.close()
                    wop = wop_ctx.enter_context(tc.tile_pool(name="wop", bufs=6))
                    wo_tiles = []
                    for ho in range(H // 512):
                        wo_sb = wop.tile([128, NQL, 512], BF16,
                                         name=f"wo_{ho}", tag="wo")
                        nc.sync.dma_start(out=wo_sb,
                                          in_=woT_r[:, :, ho * 512:(ho + 1) * 512])
                        wo_tiles.append(wo_sb)

            # ============ epilogue: normalize + o-projection ============
            ep_ctx = ExitStack()
            ep = ep_ctx.enter_context(tc.tile_pool(name="ep", bufs=1))
            eps2 = ep_ctx.enter_context(tc.tile_pool(name="eps2", bufs=2))
            rec_sb = ep.tile([1, NKVL * 256], F32)
            nc.vector.reciprocal(rec_sb, sums_sb)
            rec_bc = ep.tile([128, NKVL * 256], F32)
            nc.gpsimd.partition_broadcast(rec_bc, rec_sb, channels=128)
            attn_sb = ep.tile([128, NKVL * 256], BF16)
            nc.vector.tensor_mul(attn_sb, o_ps, rec_bc)

            for ho in range(H // 512):
                out_ps = scp.tile([KQ, 512], mybir.dt.float32,
                                  name=f"op_{ho}", tag="sc")
                for ot in range(NQL):
                    nc.tensor.matmul(out_ps, attn_sb[:, ot * 64:(ot + 1) * 64],
                                     wo_tiles[ho][:, ot, :],
                                     start=(ot == 0), stop=(ot == NQL - 1))
                out_sb = eps2.tile([KQ, 512], F32, name=f"ob_{ho}", tag="ob")
                nc.vector.tensor_copy(out_sb, out_ps)
                nc.sync.dma_start(out=out_d[:, ho * 512:(ho + 1) * 512], in_=out_sb)
            ep_ctx.close()
            wop_ctx.close()

    nc.compile()
    return nc


_prog_cache = {}


def _get_program(n_pieces):
    if n_pieces not in _prog_cache:
        _prog_cache[n_pieces] = build_program(n_pieces)
    return _prog_cache[n_pieces]


def make_in_maps(hidden_states, target_hidden, attn_mask, cos, sin,
                 Wq, Wk, Wv, Wo, q_norm_w, k_norm_w):
    """Host-side sharding/layout prep -> 8 per-core input maps."""
    B, K, _ = hidden_states.shape
    ctx_len = target_hidden.shape[1]
    L = ctx_len + K
    nlt = (L + 127) // 128
    bf = ml_dtypes.bfloat16

    # rotate-half permutation (as lhsT): rot = P @ k, pass PT = P.T
    P = np.zeros((HD, HD), np.float32)
    for i in range(HD // 2):
        P[i, i + HD // 2] = -1.0
        P[i + HD // 2, i] = 1.0
    PT = np.ascontiguousarray(P.T)

    qw = np.ascontiguousarray(
        (q_norm_w / np.sqrt(HD)).reshape(HD, 1).astype(np.float32))
    kw = np.ascontiguousarray(k_norm_w.reshape(HD, 1).astype(np.float32))

    in_maps = []
    for core in range(8):
        b, g = divmod(core, 2)
        xT = np.ascontiguousarray(target_hidden[b].T).astype(bf)
        xnT = np.ascontiguousarray(hidden_states[b].T).astype(bf)
        wkT = np.ascontiguousarray(Wk[g * DKV:(g + 1) * DKV].T).astype(bf)
        wvT = np.ascontiguousarray(Wv[g * DKV:(g + 1) * DKV].T).astype(bf)
        wqT = np.ascontiguousarray(Wq[g * DQ:(g + 1) * DQ].T).astype(bf)
        woT = np.ascontiguousarray(Wo[:, g * DQ:(g + 1) * DQ].T).astype(bf)
        csT = np.ascontiguousarray(
            np.stack([cos[b].T, sin[b].T], axis=1)).astype(np.float32)
        maskT = np.zeros((nlt * 128, K), np.float32)
        maskT[:L] = attn_mask[b, 0].T
        maskT = np.ascontiguousarray(maskT)
        in_maps.append({
            "xT": xT, "xnT": xnT, "wkT": wkT, "wvT": wvT, "wqT": wqT,
            "woT": woT, "csT": csT, "maskT": maskT, "qw": qw, "kw": kw,
            "pt": PT,
        })
    return in_maps


def kernel(hidden_states, target_hidden, attn_mask, cos, sin,
           Wq, Wk, Wv, Wo, q_norm_w, k_norm_w):
    B, K, _ = hidden_states.shape
    ctx_len = target_hidden.shape[1]
    assert ctx_len % PIECE == 0
    n_pieces = ctx_len // PIECE
    nc = _get_program(n_pieces)
    in_maps = make_in_maps(hidden_states, target_hidden, attn_mask, cos, sin,
                           Wq, Wk, Wv, Wo, q_norm_w, k_norm_w)
    res = run_bass_kernel_spmd(nc, in_maps, core_ids=list(range(8)),
                               trace=os.environ.get("KERNEL_TRACE", "0") == "1")
    out = np.zeros((B, K, H), np.float32)
    for core in range(8):
        b = core // 2
        out[b] += res.results[core]["out"]
    kernel.last_results = res
    return out
